# revision 10
# baseline (speedup 1.0000x reference)
# Distributed Trainium2 kernel for the dual-map spatial attention module:
#   x3 = x[:, :64], x2 = x[:, 64:]  (as [B, 64, N], N = 64*64 = 4096)
#   p2 = wq2 @ x2 + bq2 ; p3 = wq3 @ x3 + bq3 ; v3 = wv3 @ x3 + bv3
#   att32 = softmax(p3^T p2), att33 = softmax(p3^T p3)   (row softmax over keys)
#   out = gamma2 * (v3 @ att32^T) + gamma3 * (v3 @ att33^T) + x3
#
# Sharding: data-parallel over batch (4) x query-halves (2) -> 8 cores.
#
# v2 changes vs the original kernel:
#  - exp is split by map between the Activation engine (map32: table exp,
#    fused 1/A8 scale + -C bias, fp8e4 output) and the Vector engine
#    (map33: Schraudolph fast-exp -- energies arrive pre-scaled by
#    A8=8*log2(e) via the host-scaled query weights, so
#    exp2(x/8 - C) == bitcast_int8_to_fp8(round(x + B8)); one
#    tensor_scalar add+max per tile).  Separate psum/sbuf tiles per
#    engine: shared tiles would serialize the engines through the tile
#    tracker (both writer-writer and reader-reader chains).
#  - the out-matmuls consume the fp8 exp values in DoubleRow perf mode:
#    one matmul contracts 128 keys (2 ktiles x 64 partitions), halving
#    tensor-engine time for the out accumulation (216ns vs 432ns per
#    128-key step, measured warm).
#  - energies stay bf16 (measured: same 216ns/pair as fp8 DR); the
#    energy shift C keeps exp(E-C) inside fp8e4m3 range (max E ~6.1).
#  - prologue (projections, v3t) runs fully before the loop: jit
#    interleaving stalled both exp engines ~2us per event via the shared
#    energy-psum rings.  Projection weights are zero-padded to 64 columns
#    so the proj matmuls define every psum row (exact zero padding rows
#    in q_all without memset tricks).
#  - epilogue: accumulator staging split Act(copy)/DVE(add), normalize
#    on the otherwise-idle GPSIMD engine via a DRAM-bounce broadcast of
#    the gamma-scaled reciprocal; final chunk keeps the low-latency
#    PE-broadcast tail.
# Everything runs in the (64,128) PE tiling so row strips T0/T8 overlap.
# Steady state ~587ns per 128-key step, both exp engines ~100% busy.
import sys

if "/opt/trn_rl_repo" not in sys.path:
    sys.path.insert(0, "/opt/trn_rl_repo")

from contextlib import ExitStack

import numpy as np
import ml_dtypes

import concourse.bass as bass
import concourse.tile as tile
from concourse import bacc, mybir
from concourse.bass_utils import run_bass_kernel_spmd

BF16 = ml_dtypes.bfloat16
FP8 = ml_dtypes.float8_e4m3
dt = mybir.dt
DRM = mybir.MatmulPerfMode.DoubleRow

N = 4096          # keys per batch (64*64 spatial positions)
M_LOC = 2048      # queries per core (half a batch)
CH = 64           # output channels (c_half)
D = 8             # q/k projection dim
KA = CH + 1       # augmented contraction dim (channels + ones row)
NT = N // 128     # key tiles
NPAIR = NT // 2   # DoubleRow key-tile pairs
MC = M_LOC // 512 # query chunks per core
VST = 80          # v3t ktile stride (ch 0:65 used; stride must be %16==0)

A8 = 8.0 / np.log(2.0)      # fold into q3 projection: energies arrive as A8*E
CSHIFT = 3.0                # subtract from energies pre-exp (softmax-invariant);
                            # keeps exp(E-C) under fp8e4m3 max 240 (max E ~6.1)
B8 = 56.0 - 0.48 - A8 * CSHIFT  # schraudolph bias for int8->fp8e4 bitcast


def ts(i, size):
    return slice(i * size, (i + 1) * size)


def build(gamma2: float, gamma3: float) -> bass.Bass:
    nc = bacc.Bacc()

    x3aug = nc.declare_dram_parameter("x3aug", [KA, N], dt.bfloat16, isOutput=False)
    x2aug = nc.declare_dram_parameter("x2aug", [KA, N], dt.bfloat16, isOutput=False)
    x3q = nc.declare_dram_parameter("x3q", [KA, M_LOC], dt.bfloat16, isOutput=False)
    x3res = nc.declare_dram_parameter("x3res", [CH, M_LOC], dt.float32, isOutput=False)
    wq2bT = nc.declare_dram_parameter("wq2bT", [KA, 64], dt.bfloat16, isOutput=False)
    wq3bT = nc.declare_dram_parameter("wq3bT", [KA, 64], dt.bfloat16, isOutput=False)
    wq3qbT = nc.declare_dram_parameter("wq3qbT", [KA, 128], dt.bfloat16, isOutput=False)
    wv3bT = nc.declare_dram_parameter("wv3bT", [KA, KA], dt.bfloat16, isOutput=False)
    out_e = nc.declare_dram_parameter("out", [CH, M_LOC], dt.float32, isOutput=True)

    EXP = mybir.ActivationFunctionType.Exp
    ADD = mybir.AluOpType.add
    MAX = mybir.AluOpType.max
    NXC = 4               # x2aug/x3aug arrive in NXC column chunks
    XC = N // NXC

    with ExitStack() as ctx:
        tc = ctx.enter_context(tile.TileContext(nc))
        singles = ctx.enter_context(tc.tile_pool(name="singles", bufs=1))
        ps_e = ctx.enter_context(tc.tile_pool(name="ps_e", bufs=2, space="PSUM"))
        ps_o = ctx.enter_context(tc.tile_pool(name="ps_o", bufs=1, space="PSUM"))
        sb_e = ctx.enter_context(tc.tile_pool(name="sb_e", bufs=5))
        sb_tmp = ctx.enter_context(tc.tile_pool(name="sb_tmp", bufs=3))
        sb_out = ctx.enter_context(tc.tile_pool(name="sb_out", bufs=2))

        # ---- inputs -> SBUF, first-use order.
        wq3qbT_sb = singles.tile([KA, 128], dt.bfloat16)
        nc.sync.dma_start(out=wq3qbT_sb, in_=wq3qbT[:, :])
        x3qc = []
        for c in range(MC):
            t = singles.tile([KA, 512], dt.bfloat16, name=f"x3qc{c}", tag=f"x3qc{c}")
            if c == 0:
                nc.sync.dma_start(out=t, in_=x3q[:, ts(c, 512)])
            x3qc.append(t)
        wq2bT_sb = singles.tile([KA, 64], dt.bfloat16)
        nc.sync.dma_start(out=wq2bT_sb, in_=wq2bT[:, :])
        wq3bT_sb = singles.tile([KA, 64], dt.bfloat16)
        nc.sync.dma_start(out=wq3bT_sb, in_=wq3bT[:, :])
        wv3bT_sb = singles.tile([KA, KA], dt.bfloat16)
        nc.sync.dma_start(out=wv3bT_sb, in_=wv3bT[:, :])

        x2c = []
        x3c = []
        for c in range(NXC):
            eng = nc.gpsimd if c == 0 else nc.sync
            t2 = singles.tile([KA, XC], dt.bfloat16, name=f"x2c{c}", tag=f"x2c{c}")
            eng.dma_start(out=t2, in_=x2aug[:, ts(c, XC)])
            x2c.append(t2)
            t3 = singles.tile([KA, XC], dt.bfloat16, name=f"x3c{c}", tag=f"x3c{c}")
            eng.dma_start(out=t3, in_=x3aug[:, ts(c, XC)])
            x3c.append(t3)
        for c in range(1, MC):
            nc.sync.dma_start(out=x3qc[c], in_=x3q[:, ts(c, 512)])

        # gamma rows for the final chunk's PE-side broadcast
        cneg = singles.tile([128, 1], dt.float32)
        nc.vector.memset(cneg, -CSHIFT)
        g2row = singles.tile([1, KA], dt.bfloat16)
        nc.vector.memset(g2row, gamma2)
        g3row = singles.tile([1, KA], dt.bfloat16)
        nc.vector.memset(g3row, gamma3)

        p_all = singles.tile([128, N], dt.bfloat16)
        q_all = singles.tile([128, M_LOC], dt.bfloat16)
        # v3t: fp8 DoubleRow weights [key-in-tile, ktpair, ktile, ch];
        # ch stride padded to VST=80 for the dual-fp8 LDWEIGHTS constraint.
        v3t = singles.tile([128, NPAIR, 2, VST], dt.float8e4)

        def x_slice(tiles, j):
            per = XC // 512
            return tiles[j // per][:, ts(j % per, 512)]


        _pp_tags = ["e32", "o32a"]
        def proj_chunk(dst, j, lhs0, rhs0, lhs1, rhs1):
            # zero-padded 64-col weights write every psum row, so the cast
            # below plants exact zeros in the contraction padding rows.
            # Prologue-only: borrow the (still unused) accumulator banks to
            # deepen the ring so the PE never stalls on the cast drain.
            tag = _pp_tags[j % 2] if dst is p_all else _pp_tags[(j + 1) % 2]
            pool = ps_e if tag == "e32" else ps_o
            pp = pool.tile([128, 512], dt.float32, tag=tag)
            nc.tensor.matmul(
                pp[0:64, :], lhsT=lhs0, rhs=rhs0,
                start=True, stop=True, tile_position=(0, 0),
            )
            nc.tensor.matmul(
                pp[64:128, :], lhsT=lhs1, rhs=rhs1,
                start=True, stop=True, tile_position=(0, 64),
            )
            nc.vector.tensor_copy(out=dst[:, ts(j, 512)], in_=pp)

        _vp_tags = ["e33", "o32b", "o33a", "o33b"]
        def v3t_group(g):
            # four v3T tiles [128 keys, 65] = x3aug_tile^T @ wv3bT; cast into
            # the DoubleRow-paired fp8 layout (ntl 4g+k -> pair 2g+k//2, kt k%2)
            tag = _vp_tags[g % 4]
            pool = ps_e if tag == "e33" else ps_o
            vp = pool.tile([128, 2, 2, KA], dt.float32, tag=tag)
            per = XC // 128
            for k in range(4):
                ntl = 4 * g + k
                nc.tensor.matmul(
                    vp[:, k // 2, k % 2, :],
                    lhsT=x3c[ntl // per][:, ts(ntl % per, 128)], rhs=wv3bT_sb,
                    start=True, stop=True,
                )
            eng = nc.vector if g % 4 == 3 else nc.scalar
            eng_copy = (nc.vector.tensor_copy if g % 4 == 3 else None)
            if eng_copy is not None:
                eng_copy(out=v3t[:, 2 * g : 2 * g + 2, :, 0:KA], in_=vp)
            else:
                nc.scalar.copy(out=v3t[:, 2 * g : 2 * g + 2, :, 0:KA], in_=vp)

        def q_chunk(j):
            # the query weight is duplicated across both 64-col halves on the
            # host, so one (128,128) matmul fills all 128 psum rows at once
            tag = _pp_tags[(j + 1) % 2]
            pool = ps_e if tag == "e32" else ps_o
            pp = pool.tile([128, 512], dt.float32, tag=tag)
            nc.tensor.matmul(
                pp, lhsT=wq3qbT_sb, rhs=x3qc[j][:, :],
                start=True, stop=True, tile_position=(0, 0),
            )
            nc.vector.tensor_copy(out=q_all[:, ts(j, 512)], in_=pp)

        def p_chunk(j):
            proj_chunk(p_all, j, wq2bT_sb, x_slice(x2c, j),
                       wq3bT_sb, x_slice(x3c, j))

        # ---- epilogue machinery (identical to the original kernel)
        o_tiles = {}

        def emit_stage(mc, last=False):
            o32a, o32b, o33a, o33b = o_tiles.pop(mc)
            # stage a-halves on the Act engine, sum on DVE: splits the
            # epilogue stall across the two exp engines
            c32 = sb_tmp.tile([KA, 512], dt.float32, tag="c32")
            c33 = sb_tmp.tile([KA, 512], dt.float32, tag="c33")
            s32 = sb_tmp.tile([KA, 512], dt.float32, tag="s32")
            s33 = sb_tmp.tile([KA, 512], dt.float32, tag="s33")
            r32 = sb_tmp.tile([1, 512], dt.float32, tag="r32")
            r33 = sb_tmp.tile([1, 512], dt.float32, tag="r33")
            nc.scalar.copy(out=c32, in_=o32a)
            nc.scalar.copy(out=c33, in_=o33a)
            nc.vector.tensor_add(s32, c32, o32b)
            nc.vector.tensor_add(s33, c33, o33b)
            nc.vector.reciprocal_approx_fast(out=r32, in_=s32[0:1, :])
            nc.vector.reciprocal_approx_fast(out=r33, in_=s33[0:1, :])
            if last:
                r32b = sb_tmp.tile([1, 512], dt.bfloat16, tag="r32b")
                nc.vector.tensor_copy(out=r32b, in_=r32)
                r33b = sb_tmp.tile([1, 512], dt.bfloat16, tag="r33b")
                nc.vector.tensor_copy(out=r33b, in_=r33)
                b32p = ps_e.tile([KA, 512], dt.float32, tag="e32", name="b32p")
                nc.tensor.matmul(b32p, lhsT=g2row, rhs=r32b, start=True, stop=True)
                b33p = ps_e.tile([KA, 512], dt.float32, tag="e33", name="b33p")
                nc.tensor.matmul(b33p, lhsT=g3row, rhs=r33b, start=True, stop=True)
                return (mc, s32, s33, b32p, b33p)
            rb32 = nc.dram_tensor(f"rb32_{mc}", [1, 512], dt.float32)
            nc.gpsimd.dma_start(out=rb32[:, :], in_=r32)
            rb33 = nc.dram_tensor(f"rb33_{mc}", [1, 512], dt.float32)
            nc.gpsimd.dma_start(out=rb33[:, :], in_=r33)
            b32 = sb_tmp.tile([KA, 512], dt.float32, tag="b32")
            nc.gpsimd.dma_start(out=b32, in_=rb32[0:1, :].to_broadcast((KA, 512)))
            b33 = sb_tmp.tile([KA, 512], dt.float32, tag="b33")
            nc.gpsimd.dma_start(out=b33, in_=rb33[0:1, :].to_broadcast((KA, 512)))
            return (mc, s32, s33, b32, b33)

        def emit_norm(staged):
            mc, s32, s33, b32, b33 = staged
            t32 = sb_tmp.tile([KA, 512], dt.float32, tag="t32")
            t33 = sb_tmp.tile([KA, 512], dt.float32, tag="t33")
            if mc < MC - 1:
                # b32/b33 hold the broadcast raw reciprocal; normalize and
                # fold gamma on the idle Pool engine (the DVE's [1,512]
                # gamma-muls are gone)
                u32 = sb_tmp.tile([KA, 512], dt.float32, tag="u32")
                nc.gpsimd.tensor_mul(u32, s32, b32)
                u33 = sb_tmp.tile([KA, 512], dt.float32, tag="u33")
                nc.gpsimd.tensor_mul(u33, s33, b33)
                nc.gpsimd.tensor_scalar_mul(t32, u32, gamma2)
                nc.gpsimd.tensor_scalar_mul(t33, u33, gamma3)
                eng = nc.gpsimd
            else:
                # final chunk: gammas were folded into the PE broadcast rows
                nc.vector.tensor_mul(t32, s32, b32)
                nc.vector.tensor_mul(t33, s33, b33)
                eng = nc.vector
            s = sb_tmp.tile([KA, 512], dt.float32, tag="s")
            eng.tensor_add(s, t32, t33)
            o_sb = sb_out.tile([KA, 512], dt.float32, tag="osb")
            eng.tensor_add(o_sb, s, x3res_sb[:, ts(mc, 512)])
            nc.gpsimd.dma_start(out=out_e[:, ts(mc, 512)], in_=o_sb[1 : 1 + CH, :])

        staged = {"cur": None, "ops": []}

        def emit_out_pair(p):
            ex_a, ex_d, mc_p, j = p
            o32a, o32b, o33a, o33b = o_tiles[mc_p]
            exd8 = ex_d.bitcast(dt.float8e4)
            st, sp = (j == 0), (j == NPAIR - 1)
            for acc, lo, rhs_t in (
                (o32a, 0, ex_a), (o32b, 64, ex_a), (o33a, 0, exd8), (o33b, 64, exd8),
            ):
                nc.tensor.matmul(
                    acc,
                    lhsT=v3t[lo : lo + 64, j, :, 0:KA],
                    rhs=rhs_t[lo : lo + 64, :, :],
                    start=st, stop=sp, perf_mode=DRM, tile_position=(lo, 0),
                )
            if sp:
                staged["cur"] = emit_stage(mc_p, last=(mc_p == MC - 1))

        q_chunk(0)
        p_chunk(0)
        v3t_group(0)
        v3t_group(1)
        p_chunk(1)
        for j in range(2, N // 512):
            p_chunk(j)
            if j - 1 < MC:
                q_chunk(j - 1)
            if j < NT // 4:
                v3t_group(j)

        x3res_sb = singles.tile([KA, M_LOC], dt.float32)
        nc.vector.memset(x3res_sb[0:1, :], 0.0)
        nc.sync.dma_start(out=x3res_sb[1 : 1 + CH, :], in_=x3res[:, :])

        pend = None
        for mc in range(MC):
            o32a_t = ps_o.tile([KA, 512], dt.float32, tag="o32a")
            o32b_t = ps_o.tile([KA, 512], dt.float32, tag="o32b")
            o33a_t = ps_o.tile([KA, 512], dt.float32, tag="o33a")
            o33b_t = ps_o.tile([KA, 512], dt.float32, tag="o33b")
            o_tiles[mc] = (o32a_t, o32b_t, o33a_t, o33b_t)
            ex_a = ex_d = None
            for ntl in range(NT):
                # separate PSUM tiles per map: Act reads only e32, DVE only
                # e33 — a shared tile would chain the two engines' reads.
                e32 = ps_e.tile([128, 512], dt.float32, tag="e32")
                e33 = ps_e.tile([128, 512], dt.float32, tag="e33")
                for h, ep in ((0, e32), (1, e33)):
                    nc.tensor.matmul(
                        ep,
                        lhsT=p_all[64 * h : 64 * h + 64, ts(ntl, 128)],
                        rhs=q_all[64 * h : 64 * h + 64, ts(mc, 512)],
                        start=True, stop=True, tile_position=(64 * h, 0),
                    )
                if ntl % 2 == 0:
                    ex_a = sb_e.tile([128, 2, 512], dt.float8e4, tag="exa")
                    ex_d = sb_e.tile([128, 2, 512], dt.int8, tag="exd")
                t = ntl % 2
                # map-aligned exp split (separate tiles per engine, so the
                # writes never serialize): Act = map32 table exp (fused
                # 1/A8 scale + -C bias), DVE = map33 schraudolph int8
                nc.scalar.activation(
                    out=ex_a[:, t, :], in_=e32,
                    func=EXP, scale=float(1.0 / A8), bias=cneg[:, :],
                )
                nc.vector.tensor_scalar(
                    ex_d[:, t, :], e33,
                    float(B8), 0.0, ADD, MAX,
                )
                if ntl % 2 == 1:
                    if pend is not None:
                        emit_out_pair(pend)
                    pend = (ex_a, ex_d, mc, ntl // 2)
                if ntl == 8 and staged["cur"] is not None:
                    emit_norm(staged.pop("cur"))
                    staged["cur"] = None
        emit_out_pair(pend)
        emit_norm(staged.pop("cur"))

    nc.compile()
    return nc


_CACHE = {}


def _get_nc(gamma2: float, gamma3: float) -> bass.Bass:
    key = (gamma2, gamma3)
    if key not in _CACHE:
        _CACHE[key] = build(gamma2, gamma3)
    return _CACHE[key]


def prep(x, wq2, bq2, wq3, bq3, wv3, bv3, gamma2, gamma3):
    """Build (nc, in_maps) for the 8-core SPMD launch."""
    x = np.asarray(x, dtype=np.float32)
    B, C, W, H = x.shape
    n = W * H
    ch = C // 2
    assert (B, C, n) == (4, 128, N), (B, C, n)

    g2 = float(np.asarray(gamma2).reshape(-1)[0])
    g3 = float(np.asarray(gamma3).reshape(-1)[0])
    nc = _get_nc(g2, g3)

    def padw(w, b):
        out = np.zeros((KA, 64), np.float32)
        out[:CH, :D] = np.asarray(w, np.float32).T
        out[CH, :D] = np.asarray(b, np.float32)
        return out
    wq2bT = padw(wq2, bq2).astype(BF16)
    wq3f = padw(wq3, bq3)
    wq3bT = wq3f.astype(BF16)
    wq3q1 = wq3f * A8                   # query side carries the exp2 pre-scale
    wq3qbT = np.concatenate([wq3q1, wq3q1], axis=1).astype(BF16)
    wv3bT = np.zeros((KA, KA), np.float32)
    wv3bT[CH, 0] = 1.0
    wv3bT[:CH, 1:] = np.asarray(wv3, np.float32).T
    wv3bT[CH, 1:] = np.asarray(bv3, np.float32)
    wv3bT = wv3bT.astype(BF16)

    xf = x.reshape(B, C, n)
    ones = np.ones((1, n), np.float32)
    in_maps = []
    for b in range(B):
        x3 = xf[b, :ch]
        x2 = xf[b, ch:]
        x3aug = np.concatenate([x3, ones], axis=0).astype(BF16)
        x2aug = np.concatenate([x2, ones], axis=0).astype(BF16)
        for h in range(2):
            sl = ts(h, M_LOC)
            in_maps.append(
                {
                    "x3aug": x3aug,
                    "x2aug": x2aug,
                    "x3q": np.ascontiguousarray(x3aug[:, sl]),
                    "x3res": np.ascontiguousarray(x3[:, sl]),
                    "wq2bT": wq2bT,
                    "wq3bT": wq3bT,
                    "wq3qbT": wq3qbT,
                    "wv3bT": wv3bT,
                }
            )

    return nc, in_maps


def gather(outs, B=4, ch=CH, n=N, W=64, H=64):
    out = np.empty((B, ch, n), np.float32)
    for b in range(B):
        for h in range(2):
            out[b, :, ts(h, M_LOC)] = np.asarray(outs[2 * b + h]["out"])
    return out.reshape(B, ch, W, H)


def kernel(**inputs):
    nc, in_maps = prep(**inputs)
    res = run_bass_kernel_spmd(nc, in_maps, core_ids=list(range(8)))
    return gather(res.results)


# revision 11
# speedup vs baseline: 1.1126x; 1.1126x over previous
# Distributed Trainium2 kernel for the dual-map spatial attention module:
#   x3 = x[:, :64], x2 = x[:, 64:]  (as [B, 64, N], N = 64*64 = 4096)
#   p2 = wq2 @ x2 + bq2 ; p3 = wq3 @ x3 + bq3 ; v3 = wv3 @ x3 + bv3
#   att32 = softmax(p3^T p2), att33 = softmax(p3^T p3)   (row softmax over keys)
#   out = gamma2 * (v3 @ att32^T) + gamma3 * (v3 @ att33^T) + x3
#
# Sharding: data-parallel over batch (4) x query-halves (2) -> 8 cores.
#
# v2 changes vs the original kernel:
#  - exp is split by map between the Activation engine (map32: table exp,
#    fused 1/A8 scale + -C bias, fp8e4 output) and the Vector engine
#    (map33: Schraudolph fast-exp -- energies arrive pre-scaled by
#    A8=8*log2(e) via the host-scaled query weights, so
#    exp2(x/8 - C) == bitcast_int8_to_fp8(round(x + B8)); one
#    tensor_scalar add+max per tile).  Separate psum/sbuf tiles per
#    engine: shared tiles would serialize the engines through the tile
#    tracker (both writer-writer and reader-reader chains).
#  - the out-matmuls consume the fp8 exp values in DoubleRow perf mode:
#    one matmul contracts 128 keys (2 ktiles x 64 partitions), halving
#    tensor-engine time for the out accumulation (216ns vs 432ns per
#    128-key step, measured warm).
#  - energies stay bf16 (measured: same 216ns/pair as fp8 DR); the
#    energy shift C keeps exp(E-C) inside fp8e4m3 range (max E ~6.1).
#  - prologue (projections, v3t) runs fully before the loop: jit
#    interleaving stalled both exp engines ~2us per event via the shared
#    energy-psum rings.  Projection weights are zero-padded to 64 columns
#    so the proj matmuls define every psum row (exact zero padding rows
#    in q_all without memset tricks).
#  - epilogue: accumulator staging split Act(copy)/DVE(add), normalize
#    on the otherwise-idle GPSIMD engine via a DRAM-bounce broadcast of
#    the gamma-scaled reciprocal; final chunk keeps the low-latency
#    PE-broadcast tail.
# Everything runs in the (64,128) PE tiling so row strips T0/T8 overlap.
# Steady state ~587ns per 128-key step, both exp engines ~100% busy.
import sys

if "/opt/trn_rl_repo" not in sys.path:
    sys.path.insert(0, "/opt/trn_rl_repo")

from contextlib import ExitStack

import numpy as np
import ml_dtypes

import concourse.bass as bass
import concourse.tile as tile
from concourse import bacc, mybir
from concourse.bass_utils import run_bass_kernel_spmd

BF16 = ml_dtypes.bfloat16
FP8 = ml_dtypes.float8_e4m3
dt = mybir.dt
DRM = mybir.MatmulPerfMode.DoubleRow

N = 4096          # keys per batch (64*64 spatial positions)
M_LOC = 2048      # queries per core (half a batch)
CH = 64           # output channels (c_half)
D = 8             # q/k projection dim
KA = CH + 1       # augmented contraction dim (channels + ones row)
NT = N // 128     # key tiles
NPAIR = NT // 2   # DoubleRow key-tile pairs
MC = M_LOC // 512 # query chunks per core
VST = 80          # v3t ktile stride (ch 0:65 used; stride must be %16==0)

A8 = 8.0 / np.log(2.0)      # fold into q3 projection: energies arrive as A8*E
CSHIFT = 3.0                # subtract from energies pre-exp (softmax-invariant);
                            # keeps exp(E-C) under fp8e4m3 max 240 (max E ~6.1)
B8 = 56.0 - 0.48 - A8 * CSHIFT  # schraudolph bias for int8->fp8e4 bitcast


def ts(i, size):
    return slice(i * size, (i + 1) * size)


def build(gamma2: float, gamma3: float) -> bass.Bass:
    nc = bacc.Bacc()

    x3aug = nc.declare_dram_parameter("x3aug", [KA, N], dt.bfloat16, isOutput=False)
    x2aug = nc.declare_dram_parameter("x2aug", [KA, N], dt.bfloat16, isOutput=False)
    x3q = nc.declare_dram_parameter("x3q", [KA, M_LOC], dt.bfloat16, isOutput=False)
    x3res = nc.declare_dram_parameter("x3res", [CH, M_LOC], dt.float32, isOutput=False)
    wq2bT = nc.declare_dram_parameter("wq2bT", [KA, 64], dt.bfloat16, isOutput=False)
    wq3bT = nc.declare_dram_parameter("wq3bT", [KA, 64], dt.bfloat16, isOutput=False)
    wq3qbT = nc.declare_dram_parameter("wq3qbT", [KA, 128], dt.bfloat16, isOutput=False)
    wv3bT = nc.declare_dram_parameter("wv3bT", [KA, KA], dt.bfloat16, isOutput=False)
    out_e = nc.declare_dram_parameter("out", [CH, M_LOC], dt.float32, isOutput=True)

    EXP = mybir.ActivationFunctionType.Exp
    ADD = mybir.AluOpType.add
    MAX = mybir.AluOpType.max
    NXC = 4               # x2aug/x3aug arrive in NXC column chunks
    XC = N // NXC

    with ExitStack() as ctx:
        tc = ctx.enter_context(tile.TileContext(nc))
        singles = ctx.enter_context(tc.tile_pool(name="singles", bufs=1))
        ps_e = ctx.enter_context(tc.tile_pool(name="ps_e", bufs=2, space="PSUM"))
        ps_o = ctx.enter_context(tc.tile_pool(name="ps_o", bufs=1, space="PSUM"))
        sb_e = ctx.enter_context(tc.tile_pool(name="sb_e", bufs=5))
        sb_tmp = ctx.enter_context(tc.tile_pool(name="sb_tmp", bufs=3))
        sb_out = ctx.enter_context(tc.tile_pool(name="sb_out", bufs=2))

        # ---- inputs -> SBUF, first-use order.
        wq3qbT_sb = singles.tile([KA, 128], dt.bfloat16)
        nc.sync.dma_start(out=wq3qbT_sb, in_=wq3qbT[:, :])
        x3qc = []
        for c in range(MC):
            t = singles.tile([KA, 512], dt.bfloat16, name=f"x3qc{c}", tag=f"x3qc{c}")
            if c == 0:
                nc.sync.dma_start(out=t, in_=x3q[:, ts(c, 512)])
            x3qc.append(t)
        wq2bT_sb = singles.tile([KA, 64], dt.bfloat16)
        nc.sync.dma_start(out=wq2bT_sb, in_=wq2bT[:, :])
        wq3bT_sb = singles.tile([KA, 64], dt.bfloat16)
        nc.sync.dma_start(out=wq3bT_sb, in_=wq3bT[:, :])
        wv3bT_sb = singles.tile([KA, KA], dt.bfloat16)
        nc.sync.dma_start(out=wv3bT_sb, in_=wv3bT[:, :])

        x2c = []
        x3c = []
        for c in range(NXC):
            eng = nc.gpsimd if c == 0 else nc.sync
            t2 = singles.tile([KA, XC], dt.bfloat16, name=f"x2c{c}", tag=f"x2c{c}")
            eng.dma_start(out=t2, in_=x2aug[:, ts(c, XC)])
            x2c.append(t2)
            t3 = singles.tile([KA, XC], dt.bfloat16, name=f"x3c{c}", tag=f"x3c{c}")
            eng.dma_start(out=t3, in_=x3aug[:, ts(c, XC)])
            x3c.append(t3)
        for c in range(1, MC):
            nc.sync.dma_start(out=x3qc[c], in_=x3q[:, ts(c, 512)])

        # gamma rows for the final chunk's PE-side broadcast
        cneg = singles.tile([128, 1], dt.float32)
        nc.vector.memset(cneg, -CSHIFT)
        g2row = singles.tile([1, KA], dt.bfloat16)
        nc.vector.memset(g2row, gamma2)
        g3row = singles.tile([1, KA], dt.bfloat16)
        nc.vector.memset(g3row, gamma3)

        p_all = singles.tile([128, N], dt.bfloat16)
        q_all = singles.tile([128, M_LOC], dt.bfloat16)
        # v3t: fp8 DoubleRow weights [key-in-tile, ktpair, ktile, ch];
        # ch stride padded to VST=80 for the dual-fp8 LDWEIGHTS constraint.
        v3t = singles.tile([128, NPAIR, 2, VST], dt.float8e4)

        def x_slice(tiles, j):
            per = XC // 512
            return tiles[j // per][:, ts(j % per, 512)]


        _pp_tags = ["e32", "o32a"]
        def proj_chunk(dst, j, lhs0, rhs0, lhs1, rhs1):
            # zero-padded 64-col weights write every psum row, so the cast
            # below plants exact zeros in the contraction padding rows.
            # Prologue-only: borrow the (still unused) accumulator banks to
            # deepen the ring so the PE never stalls on the cast drain.
            tag = _pp_tags[j % 2] if dst is p_all else _pp_tags[(j + 1) % 2]
            pool = ps_e if tag == "e32" else ps_o
            pp = pool.tile([128, 512], dt.float32, tag=tag)
            nc.tensor.matmul(
                pp[0:64, :], lhsT=lhs0, rhs=rhs0,
                start=True, stop=True, tile_position=(0, 0),
            )
            nc.tensor.matmul(
                pp[64:128, :], lhsT=lhs1, rhs=rhs1,
                start=True, stop=True, tile_position=(0, 64),
            )
            nc.vector.tensor_copy(out=dst[:, ts(j, 512)], in_=pp)

        _vp_tags = ["e33", "o32b", "o33a", "o33b"]
        def v3t_group(g):
            # four v3T tiles [128 keys, 65] = x3aug_tile^T @ wv3bT; cast into
            # the DoubleRow-paired fp8 layout (ntl 4g+k -> pair 2g+k//2, kt k%2)
            tag = _vp_tags[g % 4]
            pool = ps_e if tag == "e33" else ps_o
            vp = pool.tile([128, 2, 2, KA], dt.float32, tag=tag)
            per = XC // 128
            for k in range(4):
                ntl = 4 * g + k
                nc.tensor.matmul(
                    vp[:, k // 2, k % 2, :],
                    lhsT=x3c[ntl // per][:, ts(ntl % per, 128)], rhs=wv3bT_sb,
                    start=True, stop=True,
                )
            eng = nc.vector if g % 4 == 3 else nc.scalar
            eng_copy = (nc.vector.tensor_copy if g % 4 == 3 else None)
            if eng_copy is not None:
                eng_copy(out=v3t[:, 2 * g : 2 * g + 2, :, 0:KA], in_=vp)
            else:
                nc.scalar.copy(out=v3t[:, 2 * g : 2 * g + 2, :, 0:KA], in_=vp)

        def q_chunk(j):
            # the query weight is duplicated across both 64-col halves on the
            # host, so one (128,128) matmul fills all 128 psum rows at once
            tag = _pp_tags[(j + 1) % 2]
            pool = ps_e if tag == "e32" else ps_o
            pp = pool.tile([128, 512], dt.float32, tag=tag)
            nc.tensor.matmul(
                pp, lhsT=wq3qbT_sb, rhs=x3qc[j][:, :],
                start=True, stop=True, tile_position=(0, 0),
            )
            nc.vector.tensor_copy(out=q_all[:, ts(j, 512)], in_=pp)

        def p_chunk(j):
            proj_chunk(p_all, j, wq2bT_sb, x_slice(x2c, j),
                       wq3bT_sb, x_slice(x3c, j))

        # ---- epilogue machinery (identical to the original kernel)
        o_tiles = {}

        def emit_stage(mc, last=False):
            o32a, o32b, o33a, o33b = o_tiles.pop(mc)
            # stage a-halves on the Act engine, sum on DVE: splits the
            # epilogue stall across the two exp engines
            c32 = sb_tmp.tile([KA, 512], dt.float32, tag="c32")
            c33 = sb_tmp.tile([KA, 512], dt.float32, tag="c33")
            s32 = sb_tmp.tile([KA, 512], dt.float32, tag="s32")
            s33 = sb_tmp.tile([KA, 512], dt.float32, tag="s33")
            r32 = sb_tmp.tile([1, 512], dt.float32, tag="r32")
            r33 = sb_tmp.tile([1, 512], dt.float32, tag="r33")
            nc.scalar.copy(out=c32, in_=o32a)
            nc.scalar.copy(out=c33, in_=o33a)
            nc.vector.tensor_add(s32, c32, o32b)
            nc.vector.tensor_add(s33, c33, o33b)
            nc.vector.reciprocal_approx_fast(out=r32, in_=s32[0:1, :])
            nc.vector.reciprocal_approx_fast(out=r33, in_=s33[0:1, :])
            if last:
                r32b = sb_tmp.tile([1, 512], dt.bfloat16, tag="r32b")
                nc.vector.tensor_copy(out=r32b, in_=r32)
                r33b = sb_tmp.tile([1, 512], dt.bfloat16, tag="r33b")
                nc.vector.tensor_copy(out=r33b, in_=r33)
                b32p = ps_e.tile([KA, 512], dt.float32, tag="e32", name="b32p")
                nc.tensor.matmul(b32p, lhsT=g2row, rhs=r32b, start=True, stop=True)
                b33p = ps_e.tile([KA, 512], dt.float32, tag="e33", name="b33p")
                nc.tensor.matmul(b33p, lhsT=g3row, rhs=r33b, start=True, stop=True)
                return (mc, s32, s33, b32p, b33p)
            r32g = sb_tmp.tile([1, 512], dt.float32, tag="r32g")
            nc.vector.tensor_scalar_mul(r32g, r32, gamma2)
            r33g = sb_tmp.tile([1, 512], dt.float32, tag="r33g")
            nc.vector.tensor_scalar_mul(r33g, r33, gamma3)
            rb32 = nc.dram_tensor(f"rb32_{mc}", [1, 512], dt.float32)
            nc.gpsimd.dma_start(out=rb32[:, :], in_=r32g)
            rb33 = nc.dram_tensor(f"rb33_{mc}", [1, 512], dt.float32)
            nc.gpsimd.dma_start(out=rb33[:, :], in_=r33g)
            b32 = sb_tmp.tile([KA, 512], dt.float32, tag="b32")
            nc.gpsimd.dma_start(out=b32, in_=rb32[0:1, :].to_broadcast((KA, 512)))
            b33 = sb_tmp.tile([KA, 512], dt.float32, tag="b33")
            nc.gpsimd.dma_start(out=b33, in_=rb33[0:1, :].to_broadcast((KA, 512)))
            return (mc, s32, s33, b32, b33)

        def emit_norm(staged):
            mc, s32, s33, b32, b33 = staged
            t32 = sb_tmp.tile([KA, 512], dt.float32, tag="t32")
            t33 = sb_tmp.tile([KA, 512], dt.float32, tag="t33")
            if mc < MC - 1:
                nc.gpsimd.tensor_mul(t32, s32, b32)
                nc.gpsimd.tensor_mul(t33, s33, b33)
                eng = nc.gpsimd
            else:
                # final chunk: gammas were folded into the PE broadcast rows
                nc.vector.tensor_mul(t32, s32, b32)
                nc.vector.tensor_mul(t33, s33, b33)
                eng = nc.vector
            s = sb_tmp.tile([KA, 512], dt.float32, tag="s")
            eng.tensor_add(s, t32, t33)
            o_sb = sb_out.tile([KA, 512], dt.float32, tag="osb")
            eng.tensor_add(o_sb, s, x3res_sb[:, ts(mc, 512)])
            nc.gpsimd.dma_start(out=out_e[:, ts(mc, 512)], in_=o_sb[1 : 1 + CH, :])

        staged = {"cur": None, "ops": []}

        def emit_out_pair(p):
            ex_a, ex_d, mc_p, j = p
            o32a, o32b, o33a, o33b = o_tiles[mc_p]
            exd8 = ex_d.bitcast(dt.float8e4)
            st, sp = (j == 0), (j == NPAIR - 1)
            for acc, lo, rhs_t in (
                (o32a, 0, ex_a), (o32b, 64, ex_a), (o33a, 0, exd8), (o33b, 64, exd8),
            ):
                nc.tensor.matmul(
                    acc,
                    lhsT=v3t[lo : lo + 64, j, :, 0:KA],
                    rhs=rhs_t[lo : lo + 64, :, :],
                    start=st, stop=sp, perf_mode=DRM, tile_position=(lo, 0),
                )
            if sp:
                staged["cur"] = emit_stage(mc_p, last=(mc_p == MC - 1))

        q_chunk(0)
        p_chunk(0)
        v3t_group(0)
        v3t_group(1)
        p_chunk(1)
        for j in range(2, N // 512):
            p_chunk(j)
            if j - 1 < MC:
                q_chunk(j - 1)
            if j < NT // 4:
                v3t_group(j)

        x3res_sb = singles.tile([KA, M_LOC], dt.float32)
        nc.vector.memset(x3res_sb[0:1, :], 0.0)
        nc.sync.dma_start(out=x3res_sb[1 : 1 + CH, :], in_=x3res[:, :])

        pend = None
        for mc in range(MC):
            o32a_t = ps_o.tile([KA, 512], dt.float32, tag="o32a")
            o32b_t = ps_o.tile([KA, 512], dt.float32, tag="o32b")
            o33a_t = ps_o.tile([KA, 512], dt.float32, tag="o33a")
            o33b_t = ps_o.tile([KA, 512], dt.float32, tag="o33b")
            o_tiles[mc] = (o32a_t, o32b_t, o33a_t, o33b_t)
            ex_a = ex_d = None
            for ntl in range(NT):
                # separate PSUM tiles per map: Act reads only e32, DVE only
                # e33 — a shared tile would chain the two engines' reads.
                e32 = ps_e.tile([128, 512], dt.float32, tag="e32")
                e33 = ps_e.tile([128, 512], dt.float32, tag="e33")
                for h, ep in ((0, e32), (1, e33)):
                    nc.tensor.matmul(
                        ep,
                        lhsT=p_all[64 * h : 64 * h + 64, ts(ntl, 128)],
                        rhs=q_all[64 * h : 64 * h + 64, ts(mc, 512)],
                        start=True, stop=True, tile_position=(64 * h, 0),
                    )
                if ntl % 2 == 0:
                    ex_a = sb_e.tile([128, 2, 512], dt.float8e4, tag="exa")
                    ex_d = sb_e.tile([128, 2, 512], dt.int8, tag="exd")
                t = ntl % 2
                # map-aligned exp split (separate tiles per engine, so the
                # writes never serialize): Act = map32 table exp (fused
                # 1/A8 scale + -C bias), DVE = map33 schraudolph int8
                nc.scalar.activation(
                    out=ex_a[:, t, :], in_=e32,
                    func=EXP, scale=float(1.0 / A8), bias=cneg[:, :],
                )
                nc.vector.tensor_scalar(
                    ex_d[:, t, :], e33,
                    float(B8), 0.0, ADD, MAX,
                )
                if ntl % 2 == 1:
                    if pend is not None:
                        emit_out_pair(pend)
                    pend = (ex_a, ex_d, mc, ntl // 2)
                if ntl == 8 and staged["cur"] is not None:
                    emit_norm(staged.pop("cur"))
                    staged["cur"] = None
        emit_out_pair(pend)
        emit_norm(staged.pop("cur"))

    nc.compile()
    return nc


_CACHE = {}


def _get_nc(gamma2: float, gamma3: float) -> bass.Bass:
    key = (gamma2, gamma3)
    if key not in _CACHE:
        _CACHE[key] = build(gamma2, gamma3)
    return _CACHE[key]


def prep(x, wq2, bq2, wq3, bq3, wv3, bv3, gamma2, gamma3):
    """Build (nc, in_maps) for the 8-core SPMD launch."""
    x = np.asarray(x, dtype=np.float32)
    B, C, W, H = x.shape
    n = W * H
    ch = C // 2
    assert (B, C, n) == (4, 128, N), (B, C, n)

    g2 = float(np.asarray(gamma2).reshape(-1)[0])
    g3 = float(np.asarray(gamma3).reshape(-1)[0])
    nc = _get_nc(g2, g3)

    def padw(w, b):
        out = np.zeros((KA, 64), np.float32)
        out[:CH, :D] = np.asarray(w, np.float32).T
        out[CH, :D] = np.asarray(b, np.float32)
        return out
    wq2bT = padw(wq2, bq2).astype(BF16)
    wq3f = padw(wq3, bq3)
    wq3bT = wq3f.astype(BF16)
    wq3q1 = wq3f * A8                   # query side carries the exp2 pre-scale
    wq3qbT = np.concatenate([wq3q1, wq3q1], axis=1).astype(BF16)
    wv3bT = np.zeros((KA, KA), np.float32)
    wv3bT[CH, 0] = 1.0
    wv3bT[:CH, 1:] = np.asarray(wv3, np.float32).T
    wv3bT[CH, 1:] = np.asarray(bv3, np.float32)
    wv3bT = wv3bT.astype(BF16)

    xf = x.reshape(B, C, n)
    ones = np.ones((1, n), np.float32)
    in_maps = []
    for b in range(B):
        x3 = xf[b, :ch]
        x2 = xf[b, ch:]
        x3aug = np.concatenate([x3, ones], axis=0).astype(BF16)
        x2aug = np.concatenate([x2, ones], axis=0).astype(BF16)
        for h in range(2):
            sl = ts(h, M_LOC)
            in_maps.append(
                {
                    "x3aug": x3aug,
                    "x2aug": x2aug,
                    "x3q": np.ascontiguousarray(x3aug[:, sl]),
                    "x3res": np.ascontiguousarray(x3[:, sl]),
                    "wq2bT": wq2bT,
                    "wq3bT": wq3bT,
                    "wq3qbT": wq3qbT,
                    "wv3bT": wv3bT,
                }
            )

    return nc, in_maps


def gather(outs, B=4, ch=CH, n=N, W=64, H=64):
    out = np.empty((B, ch, n), np.float32)
    for b in range(B):
        for h in range(2):
            out[b, :, ts(h, M_LOC)] = np.asarray(outs[2 * b + h]["out"])
    return out.reshape(B, ch, W, H)


def kernel(**inputs):
    nc, in_maps = prep(**inputs)
    res = run_bass_kernel_spmd(nc, in_maps, core_ids=list(range(8)))
    return gather(res.results)
